# revision 1
# baseline (speedup 1.0000x reference)
"""Trainium2 Bass kernel for 3-layer GATv2 (edge features) + global pool + MLP.

V2: bf16 edge path, chunked dma_gather (amortized SWDGE), logits in [edge,
head] orientation, DVE-leaky, alpha-broadcast messages.

Distribution: edges sharded by destination node across 8 cores (dst-sorted,
window-aligned). Node features transformed locally per shard; per-layer
all-gather of the source-side transform table (bf16); per-dst segment
softmax and message aggregation fully on-core via one-hot matmuls in PSUM.

kernel(**inputs) takes FULL inputs (as produced by the reference
setup_inputs) and returns the FULL [G, 1] output.
"""

import numpy as np

import concourse.bass as bass
import concourse.mybir as mybir
import concourse.tile as tile
from concourse import bacc, bass_utils
from concourse.bass import IndirectOffsetOnAxis
from concourse.masks import make_identity

F32 = mybir.dt.float32
BF16 = mybir.dt.float16  # HALF dtype (fp16: 8x finer mantissa than bf16)
I32 = mybir.dt.int32
I16 = mybir.dt.int16
AF = mybir.ActivationFunctionType
OP = mybir.AluOpType

# ---------------- problem constants (hardcoded per the task contract) -------
N, E, F, ED, HID, HEADS, G = 50000, 500000, 128, 6, 64, 4, 256
HC = HEADS * HID  # 256
NEG_SLOPE = 0.2
NCORES = 8
NPC = N // NCORES      # 6250 nodes per core
WIN = 128              # dst-window size (nodes)
TILE_E = 128           # edges per tile
NW = (NPC + WIN - 1) // WIN  # 49 windows per core
GB = 4                 # tiles per one-hot group
CHW = 2                # windows per gather chunk
SPLIT = 32768          # int16 gather index limit
LEAKY_ACT = False
DEBUG_DUMP = False     # add per-layer DRAM dumps (CoreSim debugging)
USE_DMA_GATHER = False  # False = per-tile indirect DMA fallback      # leaky relu on ACT (Lrelu+alpha) vs DVE stt fallback

# per layer: (k_in, c_out, H, xs_elem)  [xs_elem = gather row width, >=256B]
L_CFG = [(F, HC, HEADS, 256), (HC, HC, HEADS, 256), (HC, HID, 1, 128)]


# ---------------------------- layout helper ---------------------------------

def _layout(tA, tB):
    """Window-major tile starts + per-chunk gather metadata.

    Window-major tile order: window w owns tiles [wt0[w], wt0[w]+tA[w]+tB[w])
    with the A-segment (src < SPLIT) tiles first.  Gather order within a
    chunk: [A_w0..A_w3, B_w0..B_w3]; gcol maps window-major tile -> chunk
    gather column.  Idx cols are int16 columns (8 per tile: 128 idx / 16).
    """
    nt = [tA[w] + tB[w] for w in range(NW)]
    wt0 = np.concatenate([[0], np.cumsum(nt)[:-1]]).astype(int)
    chunks = []
    ic = 0
    for c0 in range(0, NW, CHW):
        ws = list(range(c0, min(c0 + CHW, NW)))
        ntA = sum(tA[w] for w in ws)
        ntB = sum(tB[w] for w in ws)
        gcol = {}
        ca = 0
        for w in ws:
            for i in range(tA[w]):
                gcol[int(wt0[w]) + i] = ca
                ca += 1
        cb = ntA
        for w in ws:
            for i in range(tB[w]):
                gcol[int(wt0[w]) + tA[w] + i] = cb
                cb += 1
        chunks.append(dict(ws=ws, t0=int(wt0[ws[0]]), ntc=ntA + ntB, ntA=ntA,
                           ntB=ntB, cA=ic, cB=ic + ntA * 8, gcol=gcol))
        ic += (ntA + ntB) * 8
    return wt0, chunks, ic


# ---------------------------- host-side prep --------------------------------

def _host_prep(edge_index, edge_attr):
    src = np.asarray(edge_index[0]).astype(np.int64)
    dst = np.asarray(edge_index[1]).astype(np.int64)
    order = np.argsort(dst, kind="stable")
    s_src, s_dst = src[order], dst[order]
    s_ea = np.asarray(edge_attr, dtype=np.float32)[order]

    core = s_dst // NPC
    rel = s_dst - core * NPC
    wid = rel // WIN
    if USE_DMA_GATHER:
        isB = s_src >= SPLIT
    else:
        # indirect gathers use global int32 indices; the A/B split only
        # exists for dma_gather's int16 range and wastes padded tiles
        isB = np.zeros(E, dtype=bool)

    cntA = np.zeros((NCORES, NW), dtype=np.int64)
    cntB = np.zeros((NCORES, NW), dtype=np.int64)
    np.add.at(cntA, (core[~isB], wid[~isB]), 1)
    np.add.at(cntB, (core[isB], wid[isB]), 1)
    tA = [int(x) for x in ((cntA + TILE_E - 1) // TILE_E).max(axis=0)]
    tB = [int(x) for x in ((cntB + TILE_E - 1) // TILE_E).max(axis=0)]
    for w in range(NW):
        if tA[w] + tB[w] == 0:
            tA[w] = 1
    wt0, chunks, IC = _layout(tA, tB)
    T = sum(tA) + sum(tB)

    # per-core staging arrays
    drel = np.full((NCORES, 128, T), -1.0, dtype=np.float32)
    ea_pad = np.zeros((NCORES, T * TILE_E, ED), dtype=np.float32)
    srcpad = np.zeros((NCORES, T * TILE_E), dtype=np.int64)  # global src ids

    # bucket edges per (core, window, seg)
    flat = core * (NW * 2) + wid * 2 + isB.astype(np.int64)
    bucket_order = np.argsort(flat, kind="stable")
    b_src, b_rel, b_wid, b_core = (s_src[bucket_order], rel[bucket_order],
                                   wid[bucket_order], core[bucket_order])
    b_ea = s_ea[bucket_order]
    b_isB = isB[bucket_order]
    counts = np.zeros((NCORES, NW, 2), dtype=np.int64)
    np.add.at(counts, (b_core, b_wid, b_isB.astype(np.int64)), 1)
    starts = np.zeros((NCORES, NW, 2), dtype=np.int64)
    starts.reshape(-1)[1:] = np.cumsum(counts.reshape(-1))[:-1]

    for c in range(NCORES):
        for w in range(NW):
            t0 = int(wt0[w])
            for seg in (0, 1):
                k = int(counts[c, w, seg])
                s0 = int(starts[c, w, seg])
                p0 = (t0 + (0 if seg == 0 else tA[w])) * TILE_E
                if k:
                    srcpad[c, p0:p0 + k] = b_src[s0:s0 + k]
                    ea_pad[c, p0:p0 + k] = b_ea[s0:s0 + k]
                    dv = (b_rel[s0:s0 + k] - w * WIN).astype(np.float32)
                    dcols = np.arange(p0, p0 + k)
                    drel[c, dcols % TILE_E, dcols // TILE_E] = dv

    # int16 gather indices in chunk/gather order
    idx16 = np.zeros((NCORES, 128, IC), dtype=np.int16)
    for c in range(NCORES):
        for ch in chunks:
            for t_wm, gc in ch["gcol"].items():
                vals = srcpad[c, t_wm * TILE_E:(t_wm + 1) * TILE_E].copy()
                a_side = gc < ch["ntA"]
                if not a_side:
                    vals = np.maximum(vals - SPLIT, 0)
                col0 = (ch["cA"] if a_side else ch["cB"] - ch["ntA"] * 8) + gc * 8
                v16 = vals.astype(np.int16).reshape(8, 16)  # [s, p%16]
                idx16[c, :, col0:col0 + 8] = np.tile(v16.T, (8, 1))

    # global src index per (tile, lane) for the indirect-DMA fallback
    srcTL = np.zeros((NCORES, 128, T), dtype=np.int32)
    for c in range(NCORES):
        srcTL[c] = srcpad[c].reshape(T, TILE_E).T.astype(np.int32)

    # host-precomputed one-hot S [e,(t,j)] and S^T [j,(t,e)] (bf16-able)
    jj = np.arange(TILE_E, dtype=np.float32)
    S_h = np.zeros((NCORES, 128, T * TILE_E), dtype=np.float32)
    ST_h = np.zeros((NCORES, 128, T * TILE_E), dtype=np.float32)
    for c in range(NCORES):
        S_h[c] = (drel[c][:, :, None] == jj[None, None, :]).reshape(
            128, T * TILE_E)
        dT = drel[c].T  # [T, 128e]
        ST_h[c] = (jj[:, None, None] == dT[None, :, :]).reshape(
            128, T * TILE_E)
    return S_h, ST_h, ea_pad, idx16, srcTL, tA, tB, T


def _att_blockdiag(att):
    H, C = att.shape
    bd = np.zeros((H * C, H), dtype=np.float32)
    for h in range(H):
        bd[h * C:(h + 1) * C, h] = att[h]
    return bd


def _khalf_pack(w):
    """[K, M] with K = k*128 -> [128, k*M] (row-halves side by side)."""
    K, M = w.shape
    assert K % 128 == 0
    k = K // 128
    return np.concatenate([w[q * 128:(q + 1) * 128] for q in range(k)], axis=1)


def _bf(a):
    return np.asarray(a, dtype=np.float32).astype(np.float16)


# ---------------------------- kernel builder --------------------------------

class _Cfg:
    def __init__(self, n, npc, nw, tA, tB, ncores, g):
        self.n = n
        self.npc = npc
        self.nw = nw
        self.tA = tA
        self.tB = tB
        self.wt0, self.chunks, self.IC = _layout(tA, tB)
        self.T = sum(tA) + sum(tB)
        self.ncores = ncores
        self.g = g


def _build(cfg: _Cfg):
    nc = bacc.Bacc(
        "TRN2", target_bir_lowering=False, debug=False,
        enable_asserts=False, num_devices=cfg.ncores,
    )

    npc, nw, T = cfg.npc, cfg.nw, cfg.T
    tA, tB, wt0, chunks = cfg.tA, cfg.tB, cfg.wt0, cfg.chunks
    n_nodes, g = cfg.n, cfg.g
    ntc_max = max(ch["ntc"] for ch in chunks)

    # ---- I/O declarations ----
    def din(name, shape, dt=BF16):
        return nc.dram_tensor(name, list(shape), dt, kind="ExternalInput").ap()

    xT_d = din("xT", [128, npc], F32)
    S_d = din("Soh", [128, T * TILE_E])
    ST_d = din("SToh", [128, T * TILE_E])
    if USE_DMA_GATHER:
        idx_d = din("idx16", [128, cfg.IC], I16)
    src_d = din("srcidx", [128, T], I32)
    ea_d = din("eaT", [ED, T * TILE_E])
    batch_d = din("batchw", [128, nw], F32)
    wcat_d = [din("wcat1", [128, 2 * HC], F32),
              din("wcat2", [128, 2 * 2 * HC], F32),
              din("wcat3", [128, 2 * 2 * HID], F32)]
    wedge_d = [din("wedge1", [ED, HC]), din("wedge2", [ED, HC]),
               din("wedge3", [ED, HID])]
    attbd_d = [din("attbd1", [128, 2 * HEADS], F32),
               din("attbd2", [128, 2 * HEADS], F32),
               din("attbd3", [HID, 1], F32)]
    bias_d = [din("bias1", [1, HC], F32), din("bias2", [1, HC], F32),
              din("bias3", [1, HID], F32)]
    fc1w_d = din("fc1w", [HID, HID], F32)
    fc1b_d = din("fc1b", [HID, 1], F32)
    outw_d = din("outw", [HID, 1], F32)
    outb_d = din("outb", [1, 1], F32)
    out_d = nc.dram_tensor("out", [1, g], F32, kind="ExternalOutput").ap()
    dbg_d = {}
    if DEBUG_DUMP:
        for li in range(3):
            dbg_d[f"hT{li}"] = nc.dram_tensor(
                f"dbg_hT{li}", [128, 2 * npc], F32, kind="ExternalOutput").ap()
            dbg_d[f"xd{li}"] = nc.dram_tensor(
                f"dbg_xd{li}", [128, nw * HC], BF16, kind="ExternalOutput").ap()
            # NOTE: xd stays fp16 so BF16 (=fp16 alias) matches
            dbg_d[f"acc{li}"] = nc.dram_tensor(
                f"dbg_acc{li}", [128, HC + HEADS], F32,
                kind="ExternalOutput").ap()

    with tile.TileContext(nc) as tc:
        res_pool_cm = tc.tile_pool(name="resident", bufs=1)
        res_pool = res_pool_cm.__enter__()

        def rtile(shape, dtype, name):
            return res_pool.tile(shape, dtype, tag=name, name=name)

        # ---------------- resident SBUF tensors ----------------
        hT_sb = rtile([128, 2 * npc], F32, "hT")
        xd_sb = rtile([128, nw * HC], BF16, "xd")
        h3_sb = rtile([128, nw * HID], F32, "h3")
        ident_f = rtile([128, 128], F32, "identf")
        if USE_DMA_GATHER:
            idx_sb = rtile([128, cfg.IC], I16, "idxsb")
        src_sb = rtile([128, T], I32, "srcsb")
        batch_sb = rtile([128, nw], F32, "batchsb")
        wcat_sb = [rtile([128, d.shape[1]], F32, f"wcat{i}")
                   for i, d in enumerate(wcat_d)]
        wedge_sb = [rtile([ED, d.shape[1]], BF16, f"wedge{i}")
                    for i, d in enumerate(wedge_d)]
        attbd_sb = [rtile(list(d.shape), F32, f"attbd{i}")
                    for i, d in enumerate(attbd_d)]
        bias_sb = [rtile([128, d.shape[1]], F32, f"biasm{i}")
                   for i, d in enumerate(bias_d)]
        fc1w_sb = rtile([HID, HID], F32, "fc1wsb")
        fc1b_sb = rtile([HID, 1], F32, "fc1bsb")
        outw_sb = rtile([HID, 1], F32, "outwsb")
        outb_sb = rtile([1, 1], F32, "outbsb")
        ident_bf = rtile([128, 128], BF16, "identbf")
        giota = rtile([128, g], F32, "giota")

        # loads of resident data
        nc.gpsimd.memset(xd_sb[:, :], 0.0)
        nc.gpsimd.memset(hT_sb[:, :], 0.0)
        nc.sync.dma_start(hT_sb[:, :npc], xT_d[:, :])
        if USE_DMA_GATHER:
            nc.sync.dma_start(idx_sb[:, :], idx_d[:, :])
        nc.sync.dma_start(src_sb[:, :], src_d[:, :])
        nc.sync.dma_start(batch_sb[:, :], batch_d[:, :])
        for sb, d in zip(wcat_sb + wedge_sb + attbd_sb,
                         wcat_d + wedge_d + attbd_d):
            nc.sync.dma_start(sb[:, :], d[:, :])
        for sb, d in zip([fc1w_sb, fc1b_sb, outw_sb, outb_sb],
                         [fc1w_d, fc1b_d, outw_d, outb_d]):
            nc.sync.dma_start(sb[:, :], d[:, :])
        for sb, d in zip(bias_sb, bias_d):
            nc.sync.dma_start(sb[:, :], d[0:1, :].to_broadcast([128, d.shape[1]]))

        # consts
        make_identity(nc, ident_bf[:, :])
        make_identity(nc, ident_f[:, :])
        gi_i = rtile([128, g], I32, "gi_i")
        nc.gpsimd.iota(gi_i[:, :], pattern=[[1, g]], base=0, channel_multiplier=0)
        nc.vector.tensor_copy(giota[:, :], gi_i[:, :])

        # ---------------- DRAM scratch ----------------
        with tc.tile_pool(name="dram", bufs=1, space="DRAM") as dpool:
            xs_shard_big = dpool.tile([npc, 256], BF16)
            xs_shard_small = dpool.tile([npc, 128], BF16)
            xs_full_l = [
                dpool.tile([n_nodes, 256], BF16, name="xs_full_l1"),
                dpool.tile([n_nodes, 256], BF16, name="xs_full_l2"),
                dpool.tile([n_nodes, 128], BF16, name="xs_full_l3"),
            ]
            xs_fullB_l = [
                dpool.tile([n_nodes - SPLIT, 256], BF16, name="xs_fullB_l1"),
                dpool.tile([n_nodes - SPLIT, 256], BF16, name="xs_fullB_l2"),
                dpool.tile([n_nodes - SPLIT, 128], BF16, name="xs_fullB_l3"),
            ]
            pool_in = dpool.tile([HID, g], F32)
            pool_out = dpool.tile([HID, g], F32)

            for li, (k_in, c_out, H, elem) in enumerate(L_CFG):
                khalves = k_in // 128
                chalves = (c_out + 127) // 128
                CA = c_out + H
                xs_shard = xs_shard_big if elem == 256 else xs_shard_small
                xs_full = xs_full_l[li]
                xs_fullB = xs_fullB_l[li]

                # ---------- dense phase: xd shard + xs shard ----------
                with tc.tile_pool(name=f"dps{li}", bufs=2, space="PSUM") as psd_p, \
                     tc.tile_pool(name=f"dsb{li}", bufs=3) as dsb_p:
                    for w in range(nw):
                        nn_ = min(WIN, npc - w * WIN)
                        psd = psd_p.tile([128, 2 * c_out], F32, tag="psd")
                        for q in range(khalves):
                            lhsT = hT_sb[:, q * npc + w * WIN:
                                         q * npc + w * WIN + nn_]
                            rhs = wcat_sb[li][:, q * 2 * c_out:(q + 1) * 2 * c_out]
                            nc.tensor.matmul(psd[:nn_, :], lhsT, rhs,
                                             start=(q == 0), stop=(q == khalves - 1))
                        nc.scalar.activation(
                            xd_sb[:nn_, w * c_out:(w + 1) * c_out],
                            psd[:nn_, :c_out], AF.Copy)
                        xs_stage = dsb_p.tile([128, c_out], BF16, tag="xs_stage")
                        nc.scalar.activation(xs_stage[:nn_, :],
                                             psd[:nn_, c_out:], AF.Copy)
                        nc.sync.dma_start(
                            xs_shard[w * WIN: w * WIN + nn_, :c_out],
                            xs_stage[:nn_, :])

                # ---------- all-gather xs ----------
                if cfg.ncores == 1:
                    nc.sync.dma_start(xs_full[:npc, :c_out],
                                      xs_shard[:, :c_out])
                else:
                    nc.gpsimd.collective_compute(
                        "AllGather", OP.bypass,
                        replica_groups=[list(range(cfg.ncores))],
                        ins=[xs_shard.opt()], outs=[xs_full.opt()],
                    )
                if USE_DMA_GATHER:
                    # offset-0 B table for int16 gather indices
                    nc.sync.dma_start(xs_fullB[:, :elem],
                                      xs_full[SPLIT:n_nodes, :elem])

                # ---------- edge phase ----------
                cw0 = min(128, c_out)
                with tc.tile_pool(name=f"eg{li}", bufs=2) as g_p, \
                     tc.tile_pool(name=f"ea{li}", bufs=2) as ea_p, \
                     tc.tile_pool(name=f"dr{li}", bufs=2) as dr_p, \
                     tc.tile_pool(name=f"oh{li}", bufs=3) as oh_p, \
                     tc.tile_pool(name=f"zt{li}", bufs=3) as zt_p, \
                     tc.tile_pool(name=f"al{li}", bufs=3) as al_p, \
                     tc.tile_pool(name=f"ms{li}", bufs=3) as ms_p, \
                     tc.tile_pool(name=f"fin{li}", bufs=1) as fin_p, \
                     tc.tile_pool(name=f"pt{li}", bufs=3, space="PSUM") as pt_p, \
                     tc.tile_pool(name=f"pl{li}", bufs=2, space="PSUM") as pl_p, \
                     tc.tile_pool(name=f"ph{li}", bufs=1, space="PSUM") as ph_p, \
                     tc.tile_pool(name=f"pa{li}", bufs=2, space="PSUM") as pa_p:
                    for ch in chunks:
                        t0 = ch["t0"]
                        ntc, ntA, ntB = ch["ntc"], ch["ntA"], ch["ntB"]
                        eaW = ea_p.tile([ED, ntc_max * TILE_E], BF16, tag="eaW")
                        nc.sync.dma_start(
                            eaW[:, :ntc * TILE_E],
                            ea_d[:, t0 * TILE_E:(t0 + ntc) * TILE_E])
                        Sch = oh_p.tile([128, ntc_max * TILE_E], BF16,
                                        tag="Sch")
                        nc.sync.dma_start(
                            Sch[:, :ntc * TILE_E],
                            S_d[:, t0 * TILE_E:(t0 + ntc) * TILE_E])
                        STch = dr_p.tile([128, ntc_max * TILE_E], BF16,
                                         tag="STch")
                        nc.sync.dma_start(
                            STch[:, :ntc * TILE_E],
                            ST_d[:, t0 * TILE_E:(t0 + ntc) * TILE_E])
                        xsg = g_p.tile([128, ntc_max, elem], BF16, tag="xsg")
                        if USE_DMA_GATHER:
                            if ntA:
                                nc.gpsimd.dma_gather(
                                    out_ap=xsg[:, 0:ntA, :],
                                    in_ap=xs_full[0:SPLIT, :],
                                    idxs_ap=idx_sb[:, ch["cA"]:ch["cA"] + ntA * 8],
                                    num_idxs=ntA * TILE_E,
                                    num_idxs_reg=ntA * TILE_E,
                                    elem_size=elem)
                            if ntB:
                                nc.gpsimd.dma_gather(
                                    out_ap=xsg[:, ntA:ntc, :],
                                    in_ap=xs_fullB[:, :],
                                    idxs_ap=idx_sb[:, ch["cB"]:ch["cB"] + ntB * 8],
                                    num_idxs=ntB * TILE_E,
                                    num_idxs_reg=ntB * TILE_E,
                                    elem_size=elem)
                        else:
                            for t_wm, gc in sorted(ch["gcol"].items()):
                                nc.gpsimd.indirect_dma_start(
                                    out=xsg[:, gc, :], out_offset=None,
                                    in_=xs_full[:, :],
                                    in_offset=IndirectOffsetOnAxis(
                                        ap=src_sb[:, t_wm:t_wm + 1], axis=0),
                                )

                        for w in ch["ws"]:
                            ntw = tA[w] + tB[w]
                            bt = int(wt0[w])
                            acc = pa_p.tile([128, CA], F32, tag="acc")
                            ti = 0
                            for g0 in range(0, ntw, GB):
                                gs = min(GB, ntw - g0)
                                tw = bt + g0
                                ew = gs * TILE_E
                                cols = [ch["gcol"][tw + k] for k in range(gs)]
                                co = (tw - t0) * TILE_E  # chunk col offset
                                # z^T halves + leaky
                                zT = zt_p.tile([cw0, chalves * GB * TILE_E],
                                               F32, tag="zT")
                                alpha = al_p.tile([128, GB * HEADS], BF16,
                                                  tag="alpha")
                                for q in range(chalves):
                                    cw = min(128, c_out - q * 128)
                                    tps = pt_p.tile([cw0, 512], F32, tag="tps")
                                    nc.tensor.matmul(
                                        tps[:cw, :ew],
                                        wedge_sb[li][:, q * 128:q * 128 + cw],
                                        eaW[:, (tw - t0) * TILE_E:
                                            (tw - t0) * TILE_E + ew],
                                        start=True, stop=False)
                                    nc.tensor.matmul(
                                        tps[:cw, :ew],
                                        xd_sb[:, w * c_out + q * 128:
                                              w * c_out + q * 128 + cw],
                                        STch[:, co:co + ew],
                                        start=False, stop=False)
                                    for k in range(gs):
                                        # xs^T via matmul with identity rhs
                                        # (is_transpose can't join an fp32
                                        # accumulation group from bf16 input)
                                        nc.tensor.matmul(
                                            tps[:cw, k * 128:(k + 1) * 128],
                                            xsg[:, cols[k], q * 128:q * 128 + cw],
                                            ident_bf[:, :],
                                            start=False, stop=(k == gs - 1))
                                    zsl = zT[:cw, q * GB * TILE_E:
                                             q * GB * TILE_E + ew]
                                    # leaky = 0.6 t + 0.4|t|: Abs on ACT,
                                    # one-PSUM-read stt on DVE (NCC limit)
                                    ab = zt_p.tile([cw0, GB * TILE_E], F32,
                                                   tag="ab")
                                    nc.scalar.activation(
                                        ab[:cw, :ew], tps[:cw, :ew], AF.Abs,
                                        scale=(1.0 - NEG_SLOPE) / 2)
                                    nc.vector.scalar_tensor_tensor(
                                        zsl, tps[:cw, :ew],
                                        (1.0 + NEG_SLOPE) / 2, ab[:cw, :ew],
                                        op0=OP.mult, op1=OP.add)
                                # logits [e, H] per tile (own PSUM tile,
                                # full-region start), exp per tile
                                for k in range(gs):
                                    lg = pl_p.tile([128, HEADS], F32, tag="lg")
                                    for q in range(chalves):
                                        cw = min(128, c_out - q * 128)
                                        nc.tensor.matmul(
                                            lg[:, :H],
                                            zT[:cw, q * GB * TILE_E + k * 128:
                                               q * GB * TILE_E + (k + 1) * 128],
                                            attbd_sb[li][:cw, q * H:(q + 1) * H],
                                            start=(q == 0),
                                            stop=(q == chalves - 1))
                                    nc.scalar.activation(
                                        alpha[:, k * H:(k + 1) * H],
                                        lg[:, :H], AF.Exp)
                                # messages + aug denominator columns
                                msg = ms_p.tile([128, GB, CA], BF16, tag="msg")
                                C = c_out // H
                                for k in range(gs):
                                    eng = nc.vector
                                    eng.tensor_tensor(
                                        msg[:, k, 0:c_out].rearrange(
                                            "p (h c) -> p h c", h=H),
                                        xsg[:, cols[k], 0:c_out].rearrange(
                                            "p (h c) -> p h c", h=H),
                                        alpha[:, k * H:(k + 1) * H]
                                        .to_broadcast([128, H, C]),
                                        op=OP.mult)
                                for k in range(gs):
                                    nc.vector.tensor_copy(
                                        msg[:, k, c_out:CA],
                                        alpha[:, k * H:(k + 1) * H])
                                for k in range(gs):
                                    nc.tensor.matmul(
                                        acc[:, :],
                                        Sch[:, co + k * 128:co + (k + 1) * 128],
                                        msg[:, k, :], start=(ti == 0),
                                        stop=(ti == ntw - 1))
                                    ti += 1
                            # ---- window finalize ----
                            nn_ = min(WIN, npc - w * WIN)
                            dn = fin_p.tile([128, HEADS], F32, tag="dn")
                            nc.vector.tensor_scalar_add(dn[:, :H],
                                                        acc[:, c_out:], 1e-16)
                            rcp = fin_p.tile([128, HEADS], F32, tag="rcp")
                            nc.vector.reciprocal(rcp[:, :H], dn[:, :H])
                            vv = fin_p.tile([128, 256], F32, tag="vv")
                            for h in range(H):
                                nc.vector.scalar_tensor_tensor(
                                    vv[:, h * C:(h + 1) * C],
                                    acc[:, h * C:(h + 1) * C],
                                    rcp[:, h:h + 1],
                                    bias_sb[li][:, h * C:(h + 1) * C],
                                    op0=OP.mult, op1=OP.add)
                            # elu(v) = max(v,0) + exp(min(v,0)) - 1
                            mn = fin_p.tile([128, 256], F32, tag="mn")
                            nc.vector.tensor_scalar_min(
                                mn[:, :c_out], vv[:, :c_out], 0.0)
                            em = fin_p.tile([128, 256], F32, tag="em")
                            nc.scalar.activation(em[:, :c_out], mn[:, :c_out],
                                                 AF.Exp)
                            rp = fin_p.tile([128, 256], F32, tag="rp")
                            nc.vector.tensor_scalar_max(
                                rp[:, :c_out], vv[:, :c_out], 0.0)
                            hn = fin_p.tile([128, 256], F32, tag="hn")
                            nc.vector.scalar_tensor_tensor(
                                hn[:, :c_out], em[:, :c_out], -1.0,
                                rp[:, :c_out], op0=OP.add, op1=OP.add)
                            if li < 2:
                                for q in range(chalves):
                                    htp = ph_p.tile([128, 128], F32, tag="htp")
                                    nc.tensor.matmul(
                                        htp[:, :], hn[:, q * 128:(q + 1) * 128],
                                        ident_f[:, :], start=True, stop=True)
                                    nc.scalar.activation(
                                        hT_sb[:, q * npc + w * WIN:
                                              q * npc + w * WIN + nn_],
                                        htp[:, :nn_], AF.Copy)
                            else:
                                nc.scalar.activation(
                                    h3_sb[:, w * HID:(w + 1) * HID],
                                    hn[:, :HID], AF.Copy)
                            if DEBUG_DUMP and w == 0:
                                accf = fin_p.tile([128, HC + HEADS], F32,
                                                  tag="accf")
                                nc.vector.tensor_copy(accf[:, :CA], acc[:, :])
                                nc.sync.dma_start(dbg_d[f"acc{li}"][:, :CA],
                                                  accf[:, :CA])
                if DEBUG_DUMP:
                    if li < 2:
                        nc.sync.dma_start(dbg_d[f"hT{li}"][:, :], hT_sb[:, :])
                    else:
                        nc.gpsimd.dma_start(dbg_d[f"hT{li}"][:, :nw * HID],
                                            h3_sb[:, :])
                    nc.sync.dma_start(dbg_d[f"xd{li}"][:, :nw * c_out],
                                      xd_sb[:, :nw * c_out])

            # ---------------- pooling ----------------
            with tc.tile_pool(name="poolp", bufs=2, space="PSUM") as pp_p, \
                 tc.tile_pool(name="pools", bufs=3) as ps_p:
                gps = pp_p.tile([HID, g], F32, tag="gps")
                for w in range(nw):
                    Sg = ps_p.tile([128, g], F32, tag="Sg")
                    nc.vector.tensor_tensor(
                        Sg[:, :], batch_sb[:, w:w + 1].to_broadcast([128, g]),
                        giota[:, :], op=OP.is_equal)
                    nc.tensor.matmul(gps[:, :], h3_sb[:, w * HID:(w + 1) * HID],
                                     Sg[:, :], start=(w == 0), stop=(w == nw - 1))
                gsb = ps_p.tile([HID, g], F32, tag="gsb")
                nc.vector.tensor_copy(gsb[:, :], gps[:, :])
                nc.sync.dma_start(pool_in[:, :], gsb[:, :])
                if cfg.ncores == 1:
                    nc.sync.dma_start(pool_out[:, :], pool_in[:, :])
                else:
                    nc.gpsimd.collective_compute(
                        "AllReduce", OP.add,
                        replica_groups=[list(range(cfg.ncores))],
                        ins=[pool_in.opt()], outs=[pool_out.opt()],
                    )
                pooled = ps_p.tile([HID, g], F32, tag="pooled")
                nc.sync.dma_start(pooled[:, :], pool_out[:, :])
                # fc1 + elu
                yps = pp_p.tile([HID, g], F32, tag="yps")
                nc.tensor.matmul(yps[:, :], fc1w_sb[:, :], pooled[:, :],
                                 start=True, stop=True)
                v1 = ps_p.tile([HID, g], F32, tag="v1")
                nc.vector.tensor_scalar_add(v1[:, :], yps[:, :], fc1b_sb[:, 0:1])
                mn1 = ps_p.tile([HID, g], F32, tag="mn1")
                nc.vector.tensor_scalar_min(mn1[:, :], v1[:, :], 0.0)
                em1 = ps_p.tile([HID, g], F32, tag="em1")
                nc.scalar.activation(em1[:, :], mn1[:, :], AF.Exp)
                rp1 = ps_p.tile([HID, g], F32, tag="rp1")
                nc.vector.tensor_scalar_max(rp1[:, :], v1[:, :], 0.0)
                y1 = ps_p.tile([HID, g], F32, tag="y1")
                nc.vector.scalar_tensor_tensor(
                    y1[:, :], em1[:, :], -1.0, rp1[:, :], op0=OP.add, op1=OP.add)
                # output layer
                ops_ = pp_p.tile([1, g], F32, tag="ops")
                nc.tensor.matmul(ops_[:, :], outw_sb[:, :], y1[:, :],
                                 start=True, stop=True)
                ores = ps_p.tile([1, g], F32, tag="ores")
                nc.vector.tensor_scalar_add(ores[:, :], ops_[:, :],
                                            outb_sb[0:1, 0:1])
                nc.sync.dma_start(out_d[:, :], ores[:, :])

        res_pool_cm.__exit__(None, None, None)

    nc.compile()
    return nc


# ---------------------------- public entry ----------------------------------

_CACHE = {}


def _prepare(inputs):
    S_h, ST_h, ea_pad, idx16, srcTL, tA, tB, T = _host_prep(
        inputs["edge_index"], inputs["edge_attr"])

    x = np.asarray(inputs["x"], np.float32)
    batch = np.asarray(inputs["batch"]).astype(np.int64)

    def f32(a):
        return np.ascontiguousarray(np.asarray(a, np.float32))

    wcat1 = np.concatenate([f32(inputs["w_dst1"]), f32(inputs["w_src1"])], axis=1)
    wcat2 = _khalf_pack(
        np.concatenate([f32(inputs["w_dst2"]), f32(inputs["w_src2"])], axis=1))
    wcat3 = _khalf_pack(
        np.concatenate([f32(inputs["w_dst3"]), f32(inputs["w_src3"])], axis=1))
    attbd1 = _khalf_pack(_att_blockdiag(f32(inputs["att1"])))
    attbd2 = _khalf_pack(_att_blockdiag(f32(inputs["att2"])))
    attbd3 = _att_blockdiag(f32(inputs["att3"]))  # [64, 1]

    shared = {
        "wcat1": f32(wcat1), "wcat2": f32(wcat2), "wcat3": f32(wcat3),
        "wedge1": _bf(inputs["w_edge1"]), "wedge2": _bf(inputs["w_edge2"]),
        "wedge3": _bf(inputs["w_edge3"]),
        "attbd1": f32(attbd1), "attbd2": f32(attbd2), "attbd3": f32(attbd3),
        "bias1": f32(inputs["b1"]).reshape(1, HC),
        "bias2": f32(inputs["b2"]).reshape(1, HC),
        "bias3": f32(inputs["b3"]).reshape(1, HID),
        "fc1w": f32(inputs["fc1_w"]), "fc1b": f32(inputs["fc1_b"]).reshape(HID, 1),
        "outw": f32(inputs["out_w"]), "outb": f32(inputs["out_b"]).reshape(1, 1),
    }

    in_maps = []
    for c in range(NCORES):
        xT = np.ascontiguousarray(x[c * NPC:(c + 1) * NPC].T)
        eaT = np.ascontiguousarray(ea_pad[c].T)  # [6, T*128]
        bw = np.full((128, NW), -1.0, np.float32)
        bs = batch[c * NPC:(c + 1) * NPC].astype(np.float32)
        for w in range(NW):
            nn_ = min(WIN, NPC - w * WIN)
            bw[:nn_, w] = bs[w * WIN: w * WIN + nn_]
        m = {"xT": xT, "Soh": _bf(S_h[c]), "SToh": _bf(ST_h[c]),
             "srcidx": srcTL[c], "eaT": _bf(eaT), "batchw": bw}
        if USE_DMA_GATHER:
            m["idx16"] = idx16[c]
        m.update(shared)
        in_maps.append(m)
    return in_maps, tA, tB, T


LAST_RESULT = None


def kernel(**inputs) -> np.ndarray:
    global LAST_RESULT
    import os
    in_maps, tA, tB, T = _prepare(inputs)
    key = (T, tuple(tA), tuple(tB))
    if key not in _CACHE:
        cfg = _Cfg(N, NPC, NW, tA, tB, NCORES, G)
        _CACHE[key] = _build(cfg)
    nc = _CACHE[key]
    trace = os.environ.get("GAT_TRACE", "") == "1"
    res = bass_utils.run_bass_kernel_spmd(
        nc, in_maps, core_ids=list(range(NCORES)), trace=trace)
    LAST_RESULT = res
    out = res.results[0]["out"]  # [1, G]
    return np.ascontiguousarray(out.reshape(G, 1).astype(np.float32))



# revision 29
# speedup vs baseline: 1.3120x; 1.3120x over previous
"""Trainium2 Bass kernel for 3-layer GATv2 (edge features) + global pool + MLP.

V4: chunk-batched dma_gather (one SWDGE launch per chunk side), fp16 dense
path (hT/wcat/attbd/h3), single-ACT Prelu leaky, group-batched softmax exp
written straight into the message tiles, B-side gathers from an offset slice
of the gathered table (no staging copy).

Distribution: edges sharded by destination node across 8 cores (dst-sorted,
window-aligned). Node features transformed locally per shard; per-layer
all-gather of the source-side transform table (fp16); per-dst segment
softmax and message aggregation fully on-core via one-hot matmuls in PSUM.

kernel(**inputs) takes FULL inputs (as produced by the reference
setup_inputs) and returns the FULL [G, 1] output.
"""

import numpy as np

import concourse.bass as bass
import concourse.mybir as mybir
import concourse.tile as tile
from concourse import bacc, bass_utils
from concourse.masks import make_identity

F32 = mybir.dt.float32
F32R = mybir.dt.float32r
BF16 = mybir.dt.float16  # HALF dtype (fp16)
I32 = mybir.dt.int32
I16 = mybir.dt.int16
AF = mybir.ActivationFunctionType
OP = mybir.AluOpType

# ---------------- problem constants (hardcoded per the task contract) -------
N, E, F, ED, HID, HEADS, G = 50000, 500000, 128, 6, 64, 4, 256
HC = HEADS * HID  # 256
NEG_SLOPE = 0.2
NCORES = 8
NPC = N // NCORES      # 6250 nodes per core
WIN = 128              # dst-window size (nodes)
TILE_E = 128           # edges per tile
NW = (NPC + WIN - 1) // WIN  # 49 windows per core
GB = 4                 # tiles per one-hot group
CHW = 2                # windows per gather chunk
SPLIT = 32768          # int16 gather index limit
SHARED_AG = True      # Shared addr space for AllGather outputs

# per layer: (k_in, c_out, H, xs_elem)  [xs_elem = gather row width, >=256B]
L_CFG = [(F, HC, HEADS, 256), (HC, HC, HEADS, 256), (HC, HID, 1, 128)]


# ---------------------------- layout helper ---------------------------------

def _layout(tA, tB):
    """Window-major tile starts + per-chunk gather metadata.

    Window-major tile order: window w owns tiles [wt0[w], wt0[w]+tA[w]+tB[w])
    with the A-segment (src < SPLIT) tiles first.  Gather order within a
    chunk: [A_w0..A_w1, B_w0..B_w1]; gcol maps window-major tile -> chunk
    gather column.  Idx cols are int16 columns (8 per tile: 128 idx / 16).
    """
    nt = [tA[w] + tB[w] for w in range(NW)]
    wt0 = np.concatenate([[0], np.cumsum(nt)[:-1]]).astype(int)
    chunks = []
    ic = 0
    for c0 in range(0, NW, CHW):
        ws = list(range(c0, min(c0 + CHW, NW)))
        ntA = sum(tA[w] for w in ws)
        ntB = sum(tB[w] for w in ws)
        gcol = {}
        ca = 0
        for w in ws:
            for i in range(tA[w]):
                gcol[int(wt0[w]) + i] = ca
                ca += 1
        cb = ntA
        for w in ws:
            for i in range(tB[w]):
                gcol[int(wt0[w]) + tA[w] + i] = cb
                cb += 1
        chunks.append(dict(ws=ws, t0=int(wt0[ws[0]]), ntc=ntA + ntB, ntA=ntA,
                           ntB=ntB, cA=ic, cB=ic + ntA * 8, gcol=gcol))
        ic += (ntA + ntB) * 8
    return wt0, chunks, ic


# ---------------------------- host-side prep --------------------------------

def _host_prep(edge_index, edge_attr):
    src = np.asarray(edge_index[0]).astype(np.int64)
    dst = np.asarray(edge_index[1]).astype(np.int64)
    order = np.argsort(dst, kind="stable")
    s_src, s_dst = src[order], dst[order]
    s_ea = np.asarray(edge_attr, dtype=np.float32)[order]

    core = s_dst // NPC
    rel = s_dst - core * NPC
    wid = rel // WIN
    isB = s_src >= SPLIT

    cntA = np.zeros((NCORES, NW), dtype=np.int64)
    cntB = np.zeros((NCORES, NW), dtype=np.int64)
    np.add.at(cntA, (core[~isB], wid[~isB]), 1)
    np.add.at(cntB, (core[isB], wid[isB]), 1)
    tA = [int(x) for x in ((cntA + TILE_E - 1) // TILE_E).max(axis=0)]
    tB = [int(x) for x in ((cntB + TILE_E - 1) // TILE_E).max(axis=0)]
    for w in range(NW):
        if tA[w] + tB[w] == 0:
            tA[w] = 1
    wt0, chunks, IC = _layout(tA, tB)
    T = sum(tA) + sum(tB)

    # per-core staging arrays
    drel = np.full((NCORES, 128, T), -1.0, dtype=np.float32)
    ea_pad = np.zeros((NCORES, T * TILE_E, ED), dtype=np.float32)
    srcpad = np.zeros((NCORES, T * TILE_E), dtype=np.int64)  # global src ids

    # bucket edges per (core, window, seg)
    flat = core * (NW * 2) + wid * 2 + isB.astype(np.int64)
    bucket_order = np.argsort(flat, kind="stable")
    b_src, b_rel, b_wid, b_core = (s_src[bucket_order], rel[bucket_order],
                                   wid[bucket_order], core[bucket_order])
    b_ea = s_ea[bucket_order]
    b_isB = isB[bucket_order]
    counts = np.zeros((NCORES, NW, 2), dtype=np.int64)
    np.add.at(counts, (b_core, b_wid, b_isB.astype(np.int64)), 1)
    starts = np.zeros((NCORES, NW, 2), dtype=np.int64)
    starts.reshape(-1)[1:] = np.cumsum(counts.reshape(-1))[:-1]

    for c in range(NCORES):
        for w in range(NW):
            t0 = int(wt0[w])
            for seg in (0, 1):
                k = int(counts[c, w, seg])
                s0 = int(starts[c, w, seg])
                p0 = (t0 + (0 if seg == 0 else tA[w])) * TILE_E
                if k:
                    srcpad[c, p0:p0 + k] = b_src[s0:s0 + k]
                    ea_pad[c, p0:p0 + k] = b_ea[s0:s0 + k]
                    dv = (b_rel[s0:s0 + k] - w * WIN).astype(np.float32)
                    dcols = np.arange(p0, p0 + k)
                    drel[c, dcols % TILE_E, dcols // TILE_E] = dv

    # int16 gather indices in chunk/gather order
    idx16 = np.zeros((NCORES, 128, IC), dtype=np.int16)
    for c in range(NCORES):
        for ch in chunks:
            for t_wm, gc in ch["gcol"].items():
                vals = srcpad[c, t_wm * TILE_E:(t_wm + 1) * TILE_E].copy()
                a_side = gc < ch["ntA"]
                if not a_side:
                    vals = np.maximum(vals - SPLIT, 0)
                col0 = (ch["cA"] if a_side else ch["cB"] - ch["ntA"] * 8) + gc * 8
                v16 = vals.astype(np.int16).reshape(8, 16)  # [s, p%16]
                idx16[c, :, col0:col0 + 8] = np.tile(v16.T, (8, 1))

    # host-precomputed one-hot S [e,(t,j)] and S^T [j,(t,e)]
    jj = np.arange(TILE_E, dtype=np.float32)
    S_h = np.zeros((NCORES, 128, T * TILE_E), dtype=np.float16)
    ST_h = np.zeros((NCORES, 128, T * TILE_E), dtype=np.float16)
    for c in range(NCORES):
        S_h[c] = (drel[c][:, :, None] == jj[None, None, :]).reshape(
            128, T * TILE_E)
        dT = drel[c].T  # [T, 128e]
        ST_h[c] = (jj[:, None, None] == dT[None, :, :]).reshape(
            128, T * TILE_E)
    return S_h, ST_h, ea_pad, idx16, tA, tB, T


def _att_blockdiag(att):
    H, C = att.shape
    bd = np.zeros((H * C, H), dtype=np.float32)
    for h in range(H):
        bd[h * C:(h + 1) * C, h] = att[h]
    return bd


def _khalf_pack(w):
    """[K, M] with K = k*128 -> [128, k*M] (row-halves side by side)."""
    K, M = w.shape
    assert K % 128 == 0
    k = K // 128
    return np.concatenate([w[q * 128:(q + 1) * 128] for q in range(k)], axis=1)


def _bf(a):
    return np.asarray(a, dtype=np.float32).astype(np.float16)


# ---------------------------- kernel builder --------------------------------

class _Cfg:
    def __init__(self, n, npc, nw, tA, tB, ncores, g):
        self.n = n
        self.npc = npc
        self.nw = nw
        self.tA = tA
        self.tB = tB
        self.wt0, self.chunks, self.IC = _layout(tA, tB)
        self.T = sum(tA) + sum(tB)
        self.ncores = ncores
        self.g = g


def _build(cfg: _Cfg):
    nc = bacc.Bacc(
        "TRN2", target_bir_lowering=False, debug=False,
        enable_asserts=False, num_devices=cfg.ncores,
    )

    npc, nw, T = cfg.npc, cfg.nw, cfg.T
    tA, tB, wt0, chunks = cfg.tA, cfg.tB, cfg.wt0, cfg.chunks
    n_nodes, g = cfg.n, cfg.g
    ntc_max = max(ch["ntc"] for ch in chunks)

    # ---- I/O declarations ----
    def din(name, shape, dt=BF16):
        return nc.dram_tensor(name, list(shape), dt, kind="ExternalInput").ap()

    xT_d = din("xT", [128, npc], F32)
    xTh_d = din("xTh", [128, n_nodes])
    wsrc1h_d = din("wsrc1h", [128, HC])
    S_d = din("Soh", [128, T * TILE_E])
    ST_d = din("SToh", [128, T * TILE_E])
    idx_d = din("idx16", [128, cfg.IC], I16)
    ea_d = din("eaT", [ED, T * TILE_E])
    batch_d = din("batchw", [128, nw], F32)
    wcat_d = [din("wcat1", [128, HC], F32),
              din("wcat2", [128, 2 * 2 * HC], F32),
              din("wcat3", [128, 2 * 2 * HID], F32)]
    wedge_d = [din("wedge1", [ED, HC]), din("wedge2", [ED, HC]),
               din("wedge3", [ED, HID])]
    attbd_d = [din("attbd1", [128, 2 * HEADS], F32),
               din("attbd2", [128, 2 * HEADS], F32),
               din("attbd3", [HID, 1], F32)]
    bias_d = [din("bias1", [1, HC], F32), din("bias2", [1, HC], F32),
              din("bias3", [1, HID], F32)]
    fc1w_d = din("fc1w", [HID, HID], F32)
    fc1b_d = din("fc1b", [HID, 1], F32)
    outw_d = din("outw", [HID, 1], F32)
    outb_d = din("outb", [1, 1], F32)
    out_d = nc.dram_tensor("out", [1, g], F32, kind="ExternalOutput").ap()

    with tile.TileContext(nc) as tc:
        res_pool_cm = tc.tile_pool(name="resident", bufs=1)
        res_pool = res_pool_cm.__enter__()

        def rtile(shape, dtype, name):
            return res_pool.tile(shape, dtype, tag=name, name=name)

        # ---------------- resident SBUF tensors ----------------
        hT_sb = rtile([128, 2 * npc], F32, "hT")
        xd_sb = rtile([128, nw * HC], BF16, "xd")
        h3_sb = rtile([128, nw * HID], F32, "h3")
        idx_sb = rtile([128, cfg.IC], I16, "idxsb")
        batch_sb = rtile([128, nw], F32, "batchsb")
        wcat_sb = [rtile([128, d.shape[1]], F32, f"wcat{i}")
                   for i, d in enumerate(wcat_d)]
        wedge_sb = [rtile([ED, d.shape[1]], BF16, f"wedge{i}")
                    for i, d in enumerate(wedge_d)]
        attbd_sb = [rtile(list(d.shape), F32, f"attbd{i}")
                    for i, d in enumerate(attbd_d)]
        bias_sb = [rtile([128, d.shape[1]], F32, f"biasm{i}")
                   for i, d in enumerate(bias_d)]
        fc1w_sb = rtile([HID, HID], F32, "fc1wsb")
        fc1b_sb = rtile([HID, 1], F32, "fc1bsb")
        outw_sb = rtile([HID, 1], F32, "outwsb")
        outb_sb = rtile([1, 1], F32, "outbsb")
        wsrc1h_sb = rtile([128, HC], BF16, "wsrc1h")
        ident_bf = rtile([128, 128], BF16, "identbf")
        ident_f = rtile([128, 128], F32, "identf")
        giota = rtile([128, g], F32, "giota")

        # loads of resident data
        nc.gpsimd.memset(xd_sb[:, :], 0.0)
        nc.gpsimd.memset(hT_sb[:, :], 0.0)
        nc.sync.dma_start(hT_sb[:, :npc], xT_d[:, :])
        nc.sync.dma_start(idx_sb[:, :], idx_d[:, :])
        nc.sync.dma_start(batch_sb[:, :], batch_d[:, :])
        for sb, d in zip(wcat_sb + wedge_sb + attbd_sb,
                         wcat_d + wedge_d + attbd_d):
            nc.sync.dma_start(sb[:, :], d[:, :])
        nc.sync.dma_start(wsrc1h_sb[:, :], wsrc1h_d[:, :])
        for sb, d in zip([fc1w_sb, fc1b_sb, outw_sb, outb_sb],
                         [fc1w_d, fc1b_d, outw_d, outb_d]):
            nc.sync.dma_start(sb[:, :], d[:, :])
        for sb, d in zip(bias_sb, bias_d):
            nc.sync.dma_start(sb[:, :], d[0:1, :].to_broadcast([128, d.shape[1]]))

        # consts
        make_identity(nc, ident_bf[:, :])
        make_identity(nc, ident_f[:, :])
        gi_i = rtile([128, g], I32, "gi_i")
        nc.gpsimd.iota(gi_i[:, :], pattern=[[1, g]], base=0, channel_multiplier=0)
        nc.vector.tensor_copy(giota[:, :], gi_i[:, :])

        # ---------------- DRAM scratch ----------------
        with tc.tile_pool(name="dram", bufs=1, space="DRAM") as dpool:
            aspace = "Shared" if SHARED_AG else "Local"
            xs_shard_big = dpool.tile([npc, 256], BF16)
            xs_shard_small = dpool.tile([npc, 128], BF16)
            xs_full_l = [
                dpool.tile([n_nodes, 256], BF16, name="xs_full_l1",
                           addr_space="Local"),
                dpool.tile([n_nodes, 256], BF16, name="xs_full_l2",
                           addr_space=aspace),
                dpool.tile([n_nodes, 128], BF16, name="xs_full_l3",
                           addr_space=aspace),
            ]
            pool_in = dpool.tile([HID, g], F32)
            pool_out = dpool.tile([HID, g], F32)

            pp_cm = tc.tile_pool(name="poolp", bufs=1, space="PSUM")
            ps_cm = tc.tile_pool(name="poolsg", bufs=2)
            pp_p = psg_p = gps = None

            for li, (k_in, c_out, H, elem) in enumerate(L_CFG):
                khalves = k_in // 128
                chalves = (c_out + 127) // 128
                CA = c_out + H
                xs_shard = xs_shard_big if elem == 256 else xs_shard_small
                xs_full = xs_full_l[li]

                has_src = li > 0
                wm = 2 if has_src else 1
                # ---------- dense phase: xd shard (+ xs shard for li>0) ----
                with tc.tile_pool(name=f"dps{li}", bufs=2, space="PSUM") as psd_p, \
                     tc.tile_pool(name=f"dsb{li}", bufs=(6 if li == 0 else 4)) as dsb_p:
                    if li == 0:
                        # full-table xs1 = fp16(x) @ fp16(w_src1), computed
                        # redundantly on every core -- kills the L1 AllGather.
                        # Batched: one 2048-col load + one 16-block store per
                        # chunk to amortize HWDGE per-DMA overhead.
                        XB = 2048
                        for c0 in range(0, n_nodes, XB):
                            ncol = min(XB, n_nodes - c0)
                            nblk = (ncol + 127) // 128
                            xch = dsb_p.tile([128, XB], BF16, tag="xch")
                            nc.sync.dma_start(xch[:, :ncol],
                                              xTh_d[:, c0:c0 + ncol])
                            stg = dsb_p.tile([128, XB // 128, HC], BF16,
                                             tag="stg")
                            for b in range(nblk):
                                nn_ = min(128, ncol - b * 128)
                                ps1 = psd_p.tile([128, HC], F32, tag="ps1")
                                nc.tensor.matmul(
                                    ps1[:nn_, :],
                                    xch[:, b * 128:b * 128 + nn_],
                                    wsrc1h_sb[:, :], start=True, stop=True)
                                if b % 2 == 0:
                                    nc.scalar.activation(stg[:nn_, b, :],
                                                         ps1[:nn_, :], AF.Copy)
                                else:
                                    nc.vector.tensor_copy(stg[:nn_, b, :],
                                                          ps1[:nn_, :])
                            if ncol == XB:
                                nc.sync.dma_start(
                                    xs_full[c0:c0 + ncol, :].rearrange(
                                        "(b p) c -> p b c", p=128),
                                    stg[:, :, :])
                            else:
                                for b in range(nblk):
                                    nn_ = min(128, ncol - b * 128)
                                    nc.sync.dma_start(
                                        xs_full[c0 + b * 128:
                                                c0 + b * 128 + nn_, :],
                                        stg[:nn_, b, :])
                    for w in range(nw):
                        nn_ = min(WIN, npc - w * WIN)
                        psd = psd_p.tile([128, wm * c_out], F32, tag="psd")
                        for q in range(khalves):
                            lhsT = hT_sb[:, q * npc + w * WIN:
                                         q * npc + w * WIN + nn_]
                            rhs = wcat_sb[li][:, q * wm * c_out:
                                              (q + 1) * wm * c_out]
                            nc.tensor.matmul(
                                psd[:nn_, :], lhsT, rhs,
                                start=(q == 0), stop=(q == khalves - 1))
                        nc.scalar.activation(
                            xd_sb[:nn_, w * c_out:(w + 1) * c_out],
                            psd[:nn_, :c_out], AF.Copy)
                        if has_src:
                            xs_stage = dsb_p.tile([128, c_out], BF16,
                                                  tag="xs_stage")
                            nc.vector.tensor_copy(xs_stage[:nn_, :],
                                                  psd[:nn_, c_out:])
                            nc.sync.dma_start(
                                xs_shard[w * WIN: w * WIN + nn_, :c_out],
                                xs_stage[:nn_, :c_out])

                # ---------- all-gather xs (layers 2-3) ----------
                if has_src:
                    if cfg.ncores == 1:
                        nc.sync.dma_start(xs_full[:npc, :c_out],
                                          xs_shard[:, :c_out])
                    else:
                        nc.gpsimd.collective_compute(
                            "AllGather", OP.bypass,
                            replica_groups=[list(range(cfg.ncores))],
                            ins=[xs_shard.opt()], outs=[xs_full.opt()],
                        )

                # ---------- edge phase ----------
                if li == 2:
                    pp_p = pp_cm.__enter__()
                    psg_p = ps_cm.__enter__()
                    gps = pp_p.tile([HID, g], F32, tag="gps")
                cw0 = min(128, c_out)
                with tc.tile_pool(name=f"eg{li}", bufs=2) as g_p, \
                     tc.tile_pool(name=f"ea{li}", bufs=3) as ea_p, \
                     tc.tile_pool(name=f"dr{li}", bufs=3) as dr_p, \
                     tc.tile_pool(name=f"oh{li}", bufs=3) as oh_p, \
                     tc.tile_pool(name=f"zt{li}", bufs=3) as zt_p, \
                     tc.tile_pool(name=f"ms{li}", bufs=3) as ms_p, \
                     tc.tile_pool(name=f"fin{li}", bufs=1) as fin_p, \
                     tc.tile_pool(name=f"pt{li}", bufs=(3 if li < 2 else 2), space="PSUM") as pt_p, \
                     tc.tile_pool(name=f"pl{li}", bufs=(2 if li < 2 else 1), space="PSUM") as pl_p, \
                     tc.tile_pool(name=f"ph{li}", bufs=1, space="PSUM") as ph_p, \
                     tc.tile_pool(name=f"pa{li}", bufs=2, space="PSUM") as pa_p:
                    def load_tables(ch):
                        t0, ntc = ch["t0"], ch["ntc"]
                        eaW = ea_p.tile([ED, ntc_max * TILE_E], BF16,
                                        tag="eaW")
                        nc.sync.dma_start(
                            eaW[:, :ntc * TILE_E],
                            ea_d[:, t0 * TILE_E:(t0 + ntc) * TILE_E])
                        Sch = oh_p.tile([128, ntc_max * TILE_E], BF16,
                                        tag="Sch")
                        nc.sync.dma_start(
                            Sch[:, :ntc * TILE_E],
                            S_d[:, t0 * TILE_E:(t0 + ntc) * TILE_E])
                        STch = dr_p.tile([128, ntc_max * TILE_E], BF16,
                                         tag="STch")
                        nc.sync.dma_start(
                            STch[:, :ntc * TILE_E],
                            ST_d[:, t0 * TILE_E:(t0 + ntc) * TILE_E])
                        return eaW, Sch, STch
                    prefetched = {}
                    for ci_ in range(min(2, len(chunks))):
                        prefetched[ci_] = load_tables(chunks[ci_])
                    for ci_, ch in enumerate(chunks):
                        t0 = ch["t0"]
                        ntc, ntA, ntB = ch["ntc"], ch["ntA"], ch["ntB"]
                        eaW, Sch, STch = (prefetched.pop(ci_)
                                          if ci_ in prefetched
                                          else load_tables(ch))
                        xsg = g_p.tile([128, ntc_max, elem], BF16, tag="xsg")
                        # dma_gather ucode caps at 1024 descriptors per call
                        # (measured: 2048 wedges the exec unit) -> <=8 tiles
                        MAXT = 8
                        for s0_, nt_, ic0, tbl in (
                                (0, ntA, ch["cA"], xs_full[0:SPLIT, :]),
                                (ntA, ntB, ch["cB"],
                                 xs_full[SPLIT:n_nodes, :])):
                            for b0 in range(0, nt_, MAXT):
                                nb = min(MAXT, nt_ - b0)
                                nc.gpsimd.dma_gather(
                                    out_ap=xsg[:, s0_ + b0:s0_ + b0 + nb, :],
                                    in_ap=tbl,
                                    idxs_ap=idx_sb[:, ic0 + b0 * 8:
                                                   ic0 + (b0 + nb) * 8],
                                    num_idxs=nb * TILE_E,
                                    num_idxs_reg=nb * TILE_E,
                                    elem_size=elem)

                        for w in ch["ws"]:
                            ntw = tA[w] + tB[w]
                            bt = int(wt0[w])
                            acc = pa_p.tile([128, CA], F32, tag="acc")
                            ti = 0
                            for g0 in range(0, ntw, GB):
                                gs = min(GB, ntw - g0)
                                tw = bt + g0
                                ew = gs * TILE_E
                                cols = [ch["gcol"][tw + k] for k in range(gs)]
                                co = (tw - t0) * TILE_E  # chunk col offset
                                # z^T halves; leaky via one ACT Prelu
                                zT = zt_p.tile([cw0, chalves * GB * TILE_E],
                                               F32, tag="zT")
                                msg = ms_p.tile([128, GB, CA], BF16, tag="msg")
                                for q in range(chalves):
                                    cw = min(128, c_out - q * 128)
                                    tps = pt_p.tile([cw0, 512], F32, tag="tps")
                                    nc.tensor.matmul(
                                        tps[:cw, :ew],
                                        wedge_sb[li][:, q * 128:q * 128 + cw],
                                        eaW[:, (tw - t0) * TILE_E:
                                            (tw - t0) * TILE_E + ew],
                                        start=True, stop=False)
                                    nc.tensor.matmul(
                                        tps[:cw, :ew],
                                        xd_sb[:, w * c_out + q * 128:
                                              w * c_out + q * 128 + cw],
                                        STch[:, co:co + ew],
                                        start=False, stop=False)
                                    for k in range(gs):
                                        # xs^T via matmul with identity rhs
                                        nc.tensor.matmul(
                                            tps[:cw, k * 128:(k + 1) * 128],
                                            xsg[:, cols[k], q * 128:q * 128 + cw],
                                            ident_bf[:, :],
                                            start=False, stop=(k == gs - 1))
                                    nc.scalar.activation(
                                        zT[:cw, q * GB * TILE_E:
                                           q * GB * TILE_E + ew],
                                        tps[:cw, :ew], AF.Prelu,
                                        alpha=NEG_SLOPE)
                                # logits [e, H] per tile into one PSUM tile;
                                # one exp for the whole group straight into
                                # the msg denominator columns
                                lgB = pl_p.tile([128, GB * HEADS], F32,
                                                tag="lgB")
                                for k in range(gs):
                                    for q in range(chalves):
                                        cw = min(128, c_out - q * 128)
                                        nc.tensor.matmul(
                                            lgB[:, k * H:(k + 1) * H],
                                            zT[:cw, q * GB * TILE_E + k * 128:
                                               q * GB * TILE_E + (k + 1) * 128],
                                            attbd_sb[li][:cw, q * H:(q + 1) * H],
                                            start=(q == 0),
                                            stop=(q == chalves - 1))
                                nc.scalar.activation(
                                    msg[:, 0:gs, c_out:CA],
                                    lgB[:, 0:gs * H].rearrange(
                                        "p (g h) -> p g h", g=gs),
                                    AF.Exp)
                                # messages
                                C = c_out // H
                                for k in range(gs):
                                    nc.vector.tensor_tensor(
                                        msg[:, k, 0:c_out].rearrange(
                                            "p (h c) -> p h c", h=H),
                                        xsg[:, cols[k], 0:c_out].rearrange(
                                            "p (h c) -> p h c", h=H),
                                        msg[:, k, c_out:CA]
                                        .to_broadcast([128, H, C]),
                                        op=OP.mult)
                                for k in range(gs):
                                    nc.tensor.matmul(
                                        acc[:, :],
                                        Sch[:, co + k * 128:co + (k + 1) * 128],
                                        msg[:, k, :], start=(ti == 0),
                                        stop=(ti == ntw - 1))
                                    ti += 1
                            # ---- window finalize ----
                            nn_ = min(WIN, npc - w * WIN)
                            dn = fin_p.tile([128, HEADS], F32, tag="dn")
                            nc.vector.tensor_scalar_add(dn[:, :H],
                                                        acc[:, c_out:], 1e-16)
                            rcp = fin_p.tile([128, HEADS], F32, tag="rcp")
                            nc.vector.reciprocal(rcp[:, :H], dn[:, :H])
                            vv = fin_p.tile([128, 256], F32, tag="vv")
                            for h in range(H):
                                nc.vector.scalar_tensor_tensor(
                                    vv[:, h * C:(h + 1) * C],
                                    acc[:, h * C:(h + 1) * C],
                                    rcp[:, h:h + 1],
                                    bias_sb[li][:, h * C:(h + 1) * C],
                                    op0=OP.mult, op1=OP.add)
                            # elu(v) = max(v,0) + exp(min(v,0)) - 1
                            mn = fin_p.tile([128, 256], F32, tag="mn")
                            nc.vector.tensor_scalar_min(
                                mn[:, :c_out], vv[:, :c_out], 0.0)
                            em = fin_p.tile([128, 256], F32, tag="em")
                            nc.scalar.activation(em[:, :c_out], mn[:, :c_out],
                                                 AF.Exp)
                            rp = fin_p.tile([128, 256], F32, tag="rp")
                            nc.vector.tensor_scalar_max(
                                rp[:, :c_out], vv[:, :c_out], 0.0)
                            hn = fin_p.tile([128, 256], F32, tag="hn")
                            nc.vector.scalar_tensor_tensor(
                                hn[:, :c_out], em[:, :c_out], -1.0,
                                rp[:, :c_out], op0=OP.add, op1=OP.add)
                            if li < 2:
                                for q in range(chalves):
                                    htp = ph_p.tile([128, 128], F32, tag="htp")
                                    nc.tensor.matmul(
                                        htp[:, :], hn[:, q * 128:(q + 1) * 128],
                                        ident_f[:, :], start=True, stop=True)
                                    nc.scalar.activation(
                                        hT_sb[:, q * npc + w * WIN:
                                              q * npc + w * WIN + nn_],
                                        htp[:, :nn_], AF.Copy)
                            else:
                                nc.scalar.activation(
                                    h3_sb[:, w * HID:(w + 1) * HID],
                                    hn[:, :HID], AF.Copy)
                                Sg = psg_p.tile([128, g], F32, tag="Sg")
                                nc.vector.tensor_tensor(
                                    Sg[:, :],
                                    batch_sb[:, w:w + 1].to_broadcast(
                                        [128, g]),
                                    giota[:, :], op=OP.is_equal)
                                nc.tensor.matmul(
                                    gps[:, :],
                                    h3_sb[:, w * HID:(w + 1) * HID],
                                    Sg[:, :], start=(w == 0),
                                    stop=(w == nw - 1))

            # ---------------- pooling head ----------------
            with tc.tile_pool(name="pools", bufs=2) as ps_p:
                gsb = ps_p.tile([HID, g], F32, tag="gsb")
                nc.vector.tensor_copy(gsb[:, :], gps[:, :])
                nc.sync.dma_start(pool_in[:, :], gsb[:, :])
                if cfg.ncores == 1:
                    nc.sync.dma_start(pool_out[:, :], pool_in[:, :])
                else:
                    nc.gpsimd.collective_compute(
                        "AllReduce", OP.add,
                        replica_groups=[list(range(cfg.ncores))],
                        ins=[pool_in.opt()], outs=[pool_out.opt()],
                    )
                pooled = ps_p.tile([HID, g], F32, tag="pooled")
                nc.sync.dma_start(pooled[:, :], pool_out[:, :])
                # fc1 + elu
                yps = pp_p.tile([HID, g], F32, tag="yps")
                nc.tensor.matmul(yps[:, :], fc1w_sb[:, :], pooled[:, :],
                                 start=True, stop=True)
                v1 = ps_p.tile([HID, g], F32, tag="v1")
                nc.vector.tensor_scalar_add(v1[:, :], yps[:, :], fc1b_sb[:, 0:1])
                mn1 = ps_p.tile([HID, g], F32, tag="mn1")
                nc.vector.tensor_scalar_min(mn1[:, :], v1[:, :], 0.0)
                em1 = ps_p.tile([HID, g], F32, tag="em1")
                nc.scalar.activation(em1[:, :], mn1[:, :], AF.Exp)
                rp1 = ps_p.tile([HID, g], F32, tag="rp1")
                nc.vector.tensor_scalar_max(rp1[:, :], v1[:, :], 0.0)
                y1 = ps_p.tile([HID, g], F32, tag="y1")
                nc.vector.scalar_tensor_tensor(
                    y1[:, :], em1[:, :], -1.0, rp1[:, :], op0=OP.add, op1=OP.add)
                # output layer
                ops_ = pp_p.tile([1, g], F32, tag="ops")
                nc.tensor.matmul(ops_[:, :], outw_sb[:, :], y1[:, :],
                                 start=True, stop=True)
                ores = ps_p.tile([1, g], F32, tag="ores")
                nc.vector.tensor_scalar_add(ores[:, :], ops_[:, :],
                                            outb_sb[0:1, 0:1])
                nc.sync.dma_start(out_d[:, :], ores[:, :])

            ps_cm.__exit__(None, None, None)
            pp_cm.__exit__(None, None, None)

        res_pool_cm.__exit__(None, None, None)

    nc.compile()
    return nc


# ---------------------------- public entry ----------------------------------

_CACHE = {}


def _prepare(inputs):
    S_h, ST_h, ea_pad, idx16, tA, tB, T = _host_prep(
        inputs["edge_index"], inputs["edge_attr"])

    x = np.asarray(inputs["x"], np.float32)
    batch = np.asarray(inputs["batch"]).astype(np.int64)

    def f32(a):
        return np.ascontiguousarray(np.asarray(a, np.float32))

    wcat1 = f32(inputs["w_dst1"])
    wcat2 = _khalf_pack(
        np.concatenate([f32(inputs["w_dst2"]), f32(inputs["w_src2"])], axis=1))
    wcat3 = _khalf_pack(
        np.concatenate([f32(inputs["w_dst3"]), f32(inputs["w_src3"])], axis=1))
    attbd1 = _khalf_pack(_att_blockdiag(f32(inputs["att1"])))
    attbd2 = _khalf_pack(_att_blockdiag(f32(inputs["att2"])))
    attbd3 = _att_blockdiag(f32(inputs["att3"]))  # [64, 1]

    shared = {
        "wcat1": f32(wcat1), "wcat2": f32(wcat2), "wcat3": f32(wcat3),
        "xTh": _bf(np.ascontiguousarray(np.asarray(inputs["x"],
                                                   np.float32).T)),
        "wsrc1h": _bf(inputs["w_src1"]),
        "wedge1": _bf(inputs["w_edge1"]), "wedge2": _bf(inputs["w_edge2"]),
        "wedge3": _bf(inputs["w_edge3"]),
        "attbd1": f32(attbd1), "attbd2": f32(attbd2), "attbd3": f32(attbd3),
        "bias1": f32(inputs["b1"]).reshape(1, HC),
        "bias2": f32(inputs["b2"]).reshape(1, HC),
        "bias3": f32(inputs["b3"]).reshape(1, HID),
        "fc1w": f32(inputs["fc1_w"]), "fc1b": f32(inputs["fc1_b"]).reshape(HID, 1),
        "outw": f32(inputs["out_w"]), "outb": f32(inputs["out_b"]).reshape(1, 1),
    }

    in_maps = []
    for c in range(NCORES):
        xT = np.ascontiguousarray(x[c * NPC:(c + 1) * NPC].T)
        eaT = np.ascontiguousarray(ea_pad[c].T)  # [6, T*128]
        bw = np.full((128, NW), -1.0, np.float32)
        bs = batch[c * NPC:(c + 1) * NPC].astype(np.float32)
        for w in range(NW):
            nn_ = min(WIN, NPC - w * WIN)
            bw[:nn_, w] = bs[w * WIN: w * WIN + nn_]
        m = {"xT": xT, "Soh": S_h[c], "SToh": ST_h[c],
             "idx16": idx16[c], "eaT": _bf(eaT), "batchw": bw}
        m.update(shared)
        in_maps.append(m)
    return in_maps, tA, tB, T


LAST_RESULT = None


def kernel(**inputs) -> np.ndarray:
    global LAST_RESULT
    import os
    in_maps, tA, tB, T = _prepare(inputs)
    key = (T, tuple(tA), tuple(tB))
    if key not in _CACHE:
        cfg = _Cfg(N, NPC, NW, tA, tB, NCORES, G)
        _CACHE[key] = _build(cfg)
    nc = _CACHE[key]
    trace = os.environ.get("GAT_TRACE", "") == "1"
    res = bass_utils.run_bass_kernel_spmd(
        nc, in_maps, core_ids=list(range(NCORES)), trace=trace)
    LAST_RESULT = res
    out = res.results[0]["out"]  # [1, G]
    return np.ascontiguousarray(out.reshape(G, 1).astype(np.float32))
